# revision 5
# baseline (speedup 1.0000x reference)
"""Trainium2 kernel for out = A @ W2 @ B.T with banded Gaussian W2.

Math: W2 = W1*W1 where W1[i,j] = exp(-(i-j)^2/(2*8^2)) truncated below 1e-10.
W1 > eps only for |i-j| <= 54, so in 128-blocks W2 is block-tridiagonal AND
translation-invariant: only three distinct 128x128 blocks exist (diag D0,
super-diag U = W2[j-1,j], sub-diag L = W2[j+1,j] = U.T).

Strategy (data-parallel over A's rows, 8 cores, no collectives):
  - host: fold the cheap banded product into B once per call:
    C = W2 @ B.T = (B @ W2).T (W2 symmetric), computed as a blocked
    tridiagonal multiply (~13 GFLOP, multithreaded BLAS), cast to bf16.
  - each core gets A.T slab [4096, 1024] bf16 (stationary operand layout)
    and full C [4096, 4096] bf16; the device runs ONE dense matmul
    out = A_slab @ C as 8 x 32 x 8 PSUM-accumulated 128x512 matmuls:
    all 8 PSUM banks hold the 8 m-tiles of one 512-col output chunk while
    C streams through HBM exactly once.
  - all matmuls in bf16 (fp32 PSUM accumulate) -> 1 cyc/row, FWL loads.
"""

import numpy as np

import concourse.bass as bass
import concourse.mybir as mybir
from concourse import bacc
from concourse.bass_utils import run_bass_kernel_spmd
from concourse.tile import TileContext

P = 128          # partition / block size
N = 4096         # inner dims (A cols, B rows/cols)
M_FULL = 8192    # A rows
NCORES = 8
MS = M_FULL // NCORES   # 1024 rows of A per core
NK = N // P      # 32 contraction blocks
NM = MS // P     # 8 m-tiles per core
CW = 512         # output column chunk width (= 1 PSUM bank of fp32)
NCH = N // CW    # 8 chunks

SIGMA = 8.0
TRUNC_EPS = 1e-10

_COMPILED = {}


def _bf16_dtype():
    import ml_dtypes
    return np.dtype(ml_dtypes.bfloat16)


def _w2_block(dist):
    """W2 entries for a matrix of absolute diagonal distances."""
    d = dist.astype(np.float32)
    w1 = np.exp(-(d * d) / np.float32(2.0 * SIGMA * SIGMA)).astype(np.float32)
    w1 = np.where(w1 > np.float32(TRUNC_EPS), w1, np.float32(0.0)).astype(np.float32)
    return (w1 * w1).astype(np.float32)


def _host_c_matrix(B):
    """C = W2 @ B.T = (B @ W2).T via the block-tridiagonal structure."""
    a = np.arange(P)[:, None]
    b = np.arange(P)[None, :]
    d0 = _w2_block(np.abs(a - b))          # W2[j, j]
    u = _w2_block(np.abs(a - b - P))       # W2[j-1, j]
    l = _w2_block(np.abs(P + a - b))       # W2[j+1, j]

    Bb = B.reshape(N, NK, P)               # [c, j, r]
    # D[:, j] = B[:, j] @ d0 + B[:, j-1] @ u + B[:, j+1] @ l
    D = np.matmul(Bb.transpose(1, 0, 2), d0)          # [j, c, s]
    D[1:] += np.matmul(Bb.transpose(1, 0, 2)[:-1], u)
    D[:-1] += np.matmul(Bb.transpose(1, 0, 2)[1:], l)
    # D[j, c, s] = (B @ W2)[c, j*128+s];  C = (B@W2).T -> C[j*128+s, c]
    C = D.transpose(0, 2, 1).reshape(N, N)            # [k, c]
    return np.ascontiguousarray(C.astype(_bf16_dtype()))


def _host_in_maps(A, B):
    bf16 = _bf16_dtype()
    A = np.asarray(A, dtype=np.float32)
    B = np.asarray(B, dtype=np.float32)
    assert A.shape == (M_FULL, N), A.shape
    assert B.shape == (N, N), B.shape
    a_t = np.ascontiguousarray(A.T.astype(bf16))   # [4096, 8192] bf16
    ct = _host_c_matrix(B)                         # [4096, 4096] bf16
    return [
        {
            "at": np.ascontiguousarray(a_t[:, c * MS:(c + 1) * MS]),
            "ct": ct,
        }
        for c in range(NCORES)
    ]


def _build_program(reps=1):
    """Build + compile the Bass program (one NEFF, run SPMD on 8 cores).

    reps>1 repeats the whole computation serially inside the NEFF (same
    result; used only for timing calibration).
    """
    nc = bacc.Bacc("TRN2", target_bir_lowering=False, debug=False)
    f32 = mybir.dt.float32
    bf16 = mybir.dt.bfloat16

    at_dram = nc.dram_tensor("at", [N, MS], bf16, kind="ExternalInput").ap()
    ct_dram = nc.dram_tensor("ct", [N, N], bf16, kind="ExternalInput").ap()
    out_dram = nc.dram_tensor("out", [MS, N], f32, kind="ExternalOutput").ap()

    with TileContext(nc) as tc:
        with (
            tc.tile_pool(name="atp", bufs=1) as at_pool,
            tc.tile_pool(name="ctp", bufs=10) as ct_pool,
            tc.tile_pool(name="obp", bufs=6) as ob_pool,
            tc.tile_pool(name="psp", bufs=8, space="PSUM") as ps_pool,
        ):
            for rep in range(reps):
                # A.T slab resident in SBUF ([4096, 1024] bf16 = 8 MB);
                # tiles are DMA'd lazily inside the first chunk's k-loop so
                # the at and ct streams interleave in DMA queue order.
                at_tiles = [None] * NK

                def get_at(k, rep=rep):
                    if at_tiles[k] is None:
                        at_t = at_pool.tile([P, MS], bf16, tag=f"at{k}",
                                            name=f"at_sb_{rep}_{k}")
                        nc.sync.dma_start(at_t, at_dram[k * P:(k + 1) * P, :])
                        at_tiles[k] = at_t
                    return at_tiles[k]

                # out = A_slab @ C, streamed in 512-col chunks of C
                for nu in range(NCH):
                    cs = bass.ts(nu, CW)
                    ps_o = [
                        ps_pool.tile([P, CW], f32, tag="ps",
                                     name=f"ps_o_{rep}_{nu}_{m}")
                        for m in range(NM)
                    ]
                    for k in range(NK):
                        ct_t = ct_pool.tile([P, CW], bf16, tag="ct",
                                            name=f"ct_sb_{rep}_{nu}_{k}")
                        nc.sync.dma_start(
                            ct_t, ct_dram[k * P:(k + 1) * P, cs]
                        )
                        get_at(k)
                        for m in range(NM):
                            nc.tensor.matmul(
                                ps_o[m],
                                lhsT=get_at(k)[:, m * P:(m + 1) * P],
                                rhs=ct_t,
                                start=(k == 0),
                                stop=(k == NK - 1),
                            )
                    for m in range(NM):
                        ob_t = ob_pool.tile([P, CW], f32, tag="ob",
                                            name=f"ob_sb_{rep}_{nu}_{m}")
                        if m % 2 == 0:
                            nc.vector.tensor_copy(ob_t, ps_o[m])
                        else:
                            nc.scalar.copy(ob_t, ps_o[m])
                        nc.sync.dma_start(
                            out_dram[m * P:(m + 1) * P, cs], ob_t
                        )

    nc.compile()
    return nc


def _get_program():
    if "nc" not in _COMPILED:
        _COMPILED["nc"] = _build_program()
    return _COMPILED["nc"]


def kernel(A, B):
    in_maps = _host_in_maps(A, B)
    nc = _get_program()
    res = run_bass_kernel_spmd(nc, in_maps, core_ids=list(range(NCORES)))
    return np.concatenate(
        [res.results[c]["out"] for c in range(NCORES)], axis=0
    ).astype(np.float32)


# revision 8
# speedup vs baseline: 1.0211x; 1.0211x over previous
"""Trainium2 kernel for out = A @ W2 @ B.T with banded Gaussian W2.

Math: W2 = W1*W1 where W1[i,j] = exp(-(i-j)^2/(2*8^2)) truncated below 1e-10.
W1 > eps only for |i-j| <= 54, so in 128-blocks W2 is block-tridiagonal AND
translation-invariant: only three distinct 128x128 blocks exist (diag D0,
super-diag U = W2[j-1,j], sub-diag L = W2[j+1,j] = U.T).

Strategy (data-parallel over A's rows, 8 cores, no collectives):
  - host: fold the cheap banded product into B once per call:
    C = W2 @ B.T = (B @ W2).T (W2 symmetric), computed as a blocked
    tridiagonal multiply (~13 GFLOP, multithreaded BLAS), cast to bf16.
  - each core gets A.T slab [4096, 1024] bf16 (stationary operand layout)
    and full C [4096, 4096] bf16; the device runs ONE dense matmul
    out = A_slab @ C as 8 x 32 x 8 PSUM-accumulated 128x512 matmuls.
    Each 512-col output chunk runs as two half-m groups of 4 PSUM banks,
    so consecutive chunks pipeline across the bank turnaround; C streams
    through HBM exactly once (second half reuses the chunk's SBUF tiles).
  - all matmuls in bf16 (fp32 PSUM accumulate) -> 1 cyc/row, FWL loads.
"""

import numpy as np

import concourse.bass as bass
import concourse.mybir as mybir
from concourse import bacc
from concourse.bass_utils import run_bass_kernel_spmd
from concourse.tile import TileContext

P = 128          # partition / block size
N = 4096         # inner dims (A cols, B rows/cols)
M_FULL = 8192    # A rows
NCORES = 8
MS = M_FULL // NCORES   # 1024 rows of A per core
NK = N // P      # 32 contraction blocks
NM = MS // P     # 8 m-tiles per core
CW = 512         # output column chunk width (= 1 PSUM bank of fp32)
NCH = N // CW    # 8 chunks

SIGMA = 8.0
TRUNC_EPS = 1e-10

_COMPILED = {}


def _bf16_dtype():
    import ml_dtypes
    return np.dtype(ml_dtypes.bfloat16)


def _w2_block(dist):
    """W2 entries for a matrix of absolute diagonal distances."""
    d = dist.astype(np.float32)
    w1 = np.exp(-(d * d) / np.float32(2.0 * SIGMA * SIGMA)).astype(np.float32)
    w1 = np.where(w1 > np.float32(TRUNC_EPS), w1, np.float32(0.0)).astype(np.float32)
    return (w1 * w1).astype(np.float32)


def _host_c_matrix(B):
    """C = W2 @ B.T = (B @ W2).T via the block-tridiagonal structure."""
    a = np.arange(P)[:, None]
    b = np.arange(P)[None, :]
    d0 = _w2_block(np.abs(a - b))          # W2[j, j]
    u = _w2_block(np.abs(a - b - P))       # W2[j-1, j]
    l = _w2_block(np.abs(P + a - b))       # W2[j+1, j]

    Bb = B.reshape(N, NK, P)               # [c, j, r]
    # D[:, j] = B[:, j] @ d0 + B[:, j-1] @ u + B[:, j+1] @ l
    D = np.matmul(Bb.transpose(1, 0, 2), d0)          # [j, c, s]
    D[1:] += np.matmul(Bb.transpose(1, 0, 2)[:-1], u)
    D[:-1] += np.matmul(Bb.transpose(1, 0, 2)[1:], l)
    # D[j, c, s] = (B @ W2)[c, j*128+s];  C = (B@W2).T -> C[j*128+s, c]
    C = D.transpose(0, 2, 1).reshape(N, N)            # [k, c]
    return np.ascontiguousarray(C.astype(_bf16_dtype()))


def _host_in_maps(A, B):
    bf16 = _bf16_dtype()
    A = np.asarray(A, dtype=np.float32)
    B = np.asarray(B, dtype=np.float32)
    assert A.shape == (M_FULL, N), A.shape
    assert B.shape == (N, N), B.shape
    a_t = np.ascontiguousarray(A.T.astype(bf16))   # [4096, 8192] bf16
    ct = _host_c_matrix(B)                         # [4096, 4096] bf16
    return [
        {
            "at": np.ascontiguousarray(a_t[:, c * MS:(c + 1) * MS]),
            "ct": ct,
        }
        for c in range(NCORES)
    ]


def _build_program(reps=1):
    """Build + compile the Bass program (one NEFF, run SPMD on 8 cores).

    reps>1 repeats the whole computation serially inside the NEFF (same
    result; used only for timing calibration).
    """
    nc = bacc.Bacc("TRN2", target_bir_lowering=False, debug=False)
    f32 = mybir.dt.float32
    bf16 = mybir.dt.bfloat16

    at_dram = nc.dram_tensor("at", [N, MS], bf16, kind="ExternalInput").ap()
    ct_dram = nc.dram_tensor("ct", [N, N], bf16, kind="ExternalInput").ap()
    out_dram = nc.dram_tensor("out", [MS, N], f32, kind="ExternalOutput").ap()

    with TileContext(nc) as tc:
        with (
            tc.tile_pool(name="atp", bufs=1) as at_pool,
            tc.tile_pool(name="ctp", bufs=40) as ct_pool,
            tc.tile_pool(name="obp", bufs=8) as ob_pool,
            tc.tile_pool(name="psp", bufs=8, space="PSUM") as ps_pool,
        ):
            for rep in range(reps):
                # A.T slab resident in SBUF ([4096, 1024] bf16 = 8 MB);
                # tiles are DMA'd lazily inside the first chunk's k-loop so
                # the at and ct streams interleave in DMA queue order.
                at_tiles = [None] * NK

                def get_at(k, rep=rep):
                    if at_tiles[k] is None:
                        at_t = at_pool.tile([P, MS], bf16, tag=f"at{k}",
                                            name=f"at_sb_{rep}_{k}")
                        nc.sync.dma_start(at_t, at_dram[k * P:(k + 1) * P, :])
                        at_tiles[k] = at_t
                    return at_tiles[k]

                # out = A_slab @ C, streamed in 512-col chunks of C.
                # Each chunk runs as two half-m groups of 4 PSUM banks so
                # consecutive chunks pipeline across the bank turnaround
                # (ct tiles for a chunk are DMA'd once in the first half and
                # reused from SBUF by the second).
                for nu in range(NCH):
                    cs = bass.ts(nu, CW)
                    ct_tiles = []
                    for half in range(2):
                        ms = range(half * 4, half * 4 + 4)
                        ps_o = {
                            m: ps_pool.tile([P, CW], f32, tag="ps",
                                            name=f"ps_o_{rep}_{nu}_{m}")
                            for m in ms
                        }
                        for k in range(NK):
                            if half == 0:
                                ct_t = ct_pool.tile(
                                    [P, CW], bf16, tag="ct",
                                    name=f"ct_sb_{rep}_{nu}_{k}")
                                nc.sync.dma_start(
                                    ct_t, ct_dram[k * P:(k + 1) * P, cs]
                                )
                                ct_tiles.append(ct_t)
                            else:
                                ct_t = ct_tiles[k]
                            get_at(k)
                            for m in ms:
                                nc.tensor.matmul(
                                    ps_o[m],
                                    lhsT=get_at(k)[:, m * P:(m + 1) * P],
                                    rhs=ct_t,
                                    start=(k == 0),
                                    stop=(k == NK - 1),
                                )
                        for m in ms:
                            ob_t = ob_pool.tile([P, CW], f32, tag="ob",
                                                name=f"ob_sb_{rep}_{nu}_{m}")
                            if m % 2 == 0:
                                nc.vector.tensor_copy(ob_t, ps_o[m])
                            else:
                                nc.scalar.copy(ob_t, ps_o[m])
                            nc.sync.dma_start(
                                out_dram[m * P:(m + 1) * P, cs], ob_t
                            )

    nc.compile()
    return nc


def _get_program():
    if "nc" not in _COMPILED:
        _COMPILED["nc"] = _build_program()
    return _COMPILED["nc"]


def kernel(A, B):
    in_maps = _host_in_maps(A, B)
    nc = _get_program()
    res = run_bass_kernel_spmd(nc, in_maps, core_ids=list(range(NCORES)))
    return np.concatenate(
        [res.results[c]["out"] for c in range(NCORES)], axis=0
    ).astype(np.float32)


# revision 11
# speedup vs baseline: 1.0221x; 1.0010x over previous
"""Trainium2 kernel for out = A @ W2 @ B.T with banded Gaussian W2.

Math: W2 = W1*W1 where W1[i,j] = exp(-(i-j)^2/(2*8^2)) truncated below 1e-10.
W1 > eps only for |i-j| <= 54, so in 128-blocks W2 is block-tridiagonal AND
translation-invariant: only three distinct 128x128 blocks exist (diag D0,
super-diag U = W2[j-1,j], sub-diag L = W2[j+1,j] = U.T).

Strategy (data-parallel over A's rows, 8 cores, no collectives):
  - host: fold the cheap banded product into B once per call:
    C = W2 @ B.T = (B @ W2).T (W2 symmetric), computed as a blocked
    tridiagonal multiply (~13 GFLOP, multithreaded BLAS), cast to bf16.
  - each core gets A.T slab [4096, 1024] bf16 (stationary operand layout)
    and full C [4096, 4096] bf16; the device runs ONE dense matmul
    out = A_slab @ C as 8 x 32 x 8 PSUM-accumulated 128x512 matmuls.
    Each 512-col output chunk runs as four m-groups of 2 PSUM banks, so
    consecutive groups/chunks pipeline across the bank turnaround; C
    streams through HBM exactly once (later groups reuse the chunk's
    SBUF-resident ct tiles).
  - all matmuls in bf16 (fp32 PSUM accumulate) -> 1 cyc/row, FWL loads.
"""

import numpy as np

import concourse.bass as bass
import concourse.mybir as mybir
from concourse import bacc
from concourse.bass_utils import run_bass_kernel_spmd
from concourse.tile import TileContext

P = 128          # partition / block size
N = 4096         # inner dims (A cols, B rows/cols)
M_FULL = 8192    # A rows
NCORES = 8
MS = M_FULL // NCORES   # 1024 rows of A per core
NK = N // P      # 32 contraction blocks
NM = MS // P     # 8 m-tiles per core
CW = 512         # output column chunk width (= 1 PSUM bank of fp32)
NCH = N // CW    # 8 chunks
MG = 2           # m-tiles per PSUM bank group (4 groups of 2 banks in flight)

SIGMA = 8.0
TRUNC_EPS = 1e-10

_COMPILED = {}


def _bf16_dtype():
    import ml_dtypes
    return np.dtype(ml_dtypes.bfloat16)


def _w2_block(dist):
    """W2 entries for a matrix of absolute diagonal distances."""
    d = dist.astype(np.float32)
    w1 = np.exp(-(d * d) / np.float32(2.0 * SIGMA * SIGMA)).astype(np.float32)
    w1 = np.where(w1 > np.float32(TRUNC_EPS), w1, np.float32(0.0)).astype(np.float32)
    return (w1 * w1).astype(np.float32)


def _host_c_matrix(B):
    """C = W2 @ B.T = (B @ W2).T via the block-tridiagonal structure."""
    a = np.arange(P)[:, None]
    b = np.arange(P)[None, :]
    d0 = _w2_block(np.abs(a - b))          # W2[j, j]
    u = _w2_block(np.abs(a - b - P))       # W2[j-1, j]
    l = _w2_block(np.abs(P + a - b))       # W2[j+1, j]

    Bb = B.reshape(N, NK, P)               # [c, j, r]
    # D[:, j] = B[:, j] @ d0 + B[:, j-1] @ u + B[:, j+1] @ l
    D = np.matmul(Bb.transpose(1, 0, 2), d0)          # [j, c, s]
    D[1:] += np.matmul(Bb.transpose(1, 0, 2)[:-1], u)
    D[:-1] += np.matmul(Bb.transpose(1, 0, 2)[1:], l)
    # D[j, c, s] = (B @ W2)[c, j*128+s];  C = (B@W2).T -> C[j*128+s, c]
    C = D.transpose(0, 2, 1).reshape(N, N)            # [k, c]
    return np.ascontiguousarray(C.astype(_bf16_dtype()))


def _host_in_maps(A, B):
    bf16 = _bf16_dtype()
    A = np.asarray(A, dtype=np.float32)
    B = np.asarray(B, dtype=np.float32)
    assert A.shape == (M_FULL, N), A.shape
    assert B.shape == (N, N), B.shape
    a_t = np.ascontiguousarray(A.T.astype(bf16))   # [4096, 8192] bf16
    ct = _host_c_matrix(B)                         # [4096, 4096] bf16
    return [
        {
            "at": np.ascontiguousarray(a_t[:, c * MS:(c + 1) * MS]),
            "ct": ct,
        }
        for c in range(NCORES)
    ]


def _build_program(reps=1):
    """Build + compile the Bass program (one NEFF, run SPMD on 8 cores).

    reps>1 repeats the whole computation serially inside the NEFF (same
    result; used only for timing calibration).
    """
    nc = bacc.Bacc("TRN2", target_bir_lowering=False, debug=False)
    f32 = mybir.dt.float32
    bf16 = mybir.dt.bfloat16

    at_dram = nc.dram_tensor("at", [N, MS], bf16, kind="ExternalInput").ap()
    ct_dram = nc.dram_tensor("ct", [N, N], bf16, kind="ExternalInput").ap()
    out_dram = nc.dram_tensor("out", [MS, N], f32, kind="ExternalOutput").ap()

    with TileContext(nc) as tc:
        with (
            tc.tile_pool(name="atp", bufs=1) as at_pool,
            tc.tile_pool(name="ctp", bufs=40) as ct_pool,
            tc.tile_pool(name="obp", bufs=8) as ob_pool,
            tc.tile_pool(name="psp", bufs=8, space="PSUM") as ps_pool,
        ):
            for rep in range(reps):
                # A.T slab resident in SBUF ([4096, 1024] bf16 = 8 MB);
                # tiles are DMA'd lazily inside the first chunk's k-loop so
                # the at and ct streams interleave in DMA queue order.
                at_tiles = [None] * NK

                def get_at(k, rep=rep):
                    if at_tiles[k] is None:
                        at_t = at_pool.tile([P, MS], bf16, tag=f"at{k}",
                                            name=f"at_sb_{rep}_{k}")
                        nc.sync.dma_start(at_t, at_dram[k * P:(k + 1) * P, :])
                        at_tiles[k] = at_t
                    return at_tiles[k]

                # out = A_slab @ C, streamed in 512-col chunks of C.
                # Each chunk runs as NM/MG m-groups of MG PSUM banks so
                # consecutive chunks/groups pipeline across the bank
                # turnaround (ct tiles for a chunk are DMA'd once in the
                # first group and reused from SBUF by the rest).
                for nu in range(NCH):
                    cs = bass.ts(nu, CW)
                    ct_tiles = []
                    for g in range(NM // MG):
                        ms = range(g * MG, (g + 1) * MG)
                        ps_o = {
                            m: ps_pool.tile([P, CW], f32, tag="ps",
                                            name=f"ps_o_{rep}_{nu}_{m}")
                            for m in ms
                        }
                        for k in range(NK):
                            if g == 0:
                                ct_t = ct_pool.tile(
                                    [P, CW], bf16, tag="ct",
                                    name=f"ct_sb_{rep}_{nu}_{k}")
                                nc.sync.dma_start(
                                    ct_t, ct_dram[k * P:(k + 1) * P, cs]
                                )
                                ct_tiles.append(ct_t)
                            else:
                                ct_t = ct_tiles[k]
                            get_at(k)
                            for m in ms:
                                nc.tensor.matmul(
                                    ps_o[m],
                                    lhsT=get_at(k)[:, m * P:(m + 1) * P],
                                    rhs=ct_t,
                                    start=(k == 0),
                                    stop=(k == NK - 1),
                                )
                        for m in ms:
                            ob_t = ob_pool.tile([P, CW], f32, tag="ob",
                                                name=f"ob_sb_{rep}_{nu}_{m}")
                            if m % 2 == 0:
                                nc.vector.tensor_copy(ob_t, ps_o[m])
                            else:
                                nc.scalar.copy(ob_t, ps_o[m])
                            nc.sync.dma_start(
                                out_dram[m * P:(m + 1) * P, cs], ob_t
                            )

    nc.compile()
    return nc


def _get_program():
    if "nc" not in _COMPILED:
        _COMPILED["nc"] = _build_program()
    return _COMPILED["nc"]


def kernel(A, B):
    in_maps = _host_in_maps(A, B)
    nc = _get_program()
    res = run_bass_kernel_spmd(nc, in_maps, core_ids=list(range(NCORES)))
    return np.concatenate(
        [res.results[c]["out"] for c in range(NCORES)], axis=0
    ).astype(np.float32)
